# revision 19
# baseline (speedup 1.0000x reference)
"""LoRA Linear kernel for Trainium2, 8 NeuronCores.

Computes out = x @ (W + lora_A @ lora_B)^T + bias for
x [4, 2048, 4096], W [4096, 4096], lora_A [4096, 16], lora_B [16, 4096].

Sharding: 2-way over tokens (M = 8192 -> 4096/core) x 4-way over
out_features (4096 -> 1024/core). The rank-16 LoRA delta is folded into
the weight during input marshaling (W_tot = W + A@B, 0.2% of the FLOPs;
the 274.9 GFLOP GEMM runs on device).

Mixed precision: the first 24 of 32 contraction tiles run in bf16; the
last 8 run as 4 double-pumped fp8(e4m3) DoubleRow matmuls (two k-tiles
per PE pass), cutting PE time ~11% while keeping rel_l2 ~1.9e-2.
Operands are pre-scaled on the host (x*16, W*4096, bias*65536 - exact
power-of-2 scales) so the e4m3 values sit mid-range and the fp8 and
bf16 partial products accumulate in one PSUM group; the 2^-16 descale
rides the host-side output upcast for free.

Device schedule: W_tot^T streams into a resident SBUF tile slice by
slice while the first four token tiles run as a k-major wavefront, so
the PE is saturated from ~13us on; the remaining 28 token tiles then
run back-to-back. PSUM tiles span two banks ([128, 1024] f32, 4 in
rotation = all 8 banks) so each token tile needs a single DVE bias-add
and a single 2KB-per-partition bf16 store.
"""

import ml_dtypes

import numpy as np

import concourse.bass as bass
import concourse.bacc as bacc
import concourse.mybir as mybir
import concourse.tile as tile
from concourse.bass_utils import run_bass_kernel_spmd

IN_F = 4096
OUT_F = 4096
RANK = 16
BATCH, SEQ = 4, 2048
M_TOT = BATCH * SEQ          # 8192 tokens
MG, OG = 2, 4                # shard grid: token-groups x outfeature-groups
M_LOC = M_TOT // MG          # 4096 tokens per core
O_LOC = OUT_F // OG          # 1024 out features per core
P = 128
KI = IN_F // P               # 32 contraction tiles
KB = 24                      # contraction tiles done in bf16
NP8 = (KI - KB) // 2         # fp8 DoubleRow pairs (4 -> 8 k-tiles)
K8 = KB * P                  # first fp8-region k index (3072)
NF = 512                     # matmul moving free dim (one PSUM bank)
OS = O_LOC // NF             # 2 output column passes
MT = M_LOC // P              # 32 token tiles per core
NLEAD = 4                    # token tiles in the k-major lead wavefront
XCH = 4                      # k-chunks per lead x-tile DMA
SX = 16.0                    # fp8 scale for x
SW = 4096.0                  # fp8 scale for W
SOUT = SX * SW               # psum / bias / output scale

F32 = mybir.dt.float32
BF16 = mybir.dt.bfloat16
FP8 = mybir.dt.float8e4
E4M3 = ml_dtypes.float8_e4m3

_cache = {}


def _build():
    nc = bacc.Bacc(None, target_bir_lowering=False)

    # x pre-tiled on host to [MT, P, KB, P]: (mt, i_within, i_tile, m)
    xt = nc.dram_tensor("xt", [MT, P, KB, P], BF16, kind="ExternalInput")
    # fp8 lanes: (mt, i_within, pair, lane, m)
    x8t = nc.dram_tensor("x8t", [MT, P, NP8, 2, P], FP8, kind="ExternalInput")
    # pre-folded, pre-scaled (W + lora_A @ lora_B)^T column shard
    wt = nc.dram_tensor("wt", [KB * P, O_LOC], BF16, kind="ExternalInput")
    w8 = nc.dram_tensor("w8", [P, NP8, 2, O_LOC], FP8, kind="ExternalInput")
    br = nc.dram_tensor("br", [P, O_LOC], F32, kind="ExternalInput")
    out = nc.dram_tensor("out", [M_LOC, O_LOC], BF16, kind="ExternalOutput")

    with tile.TileContext(nc) as tc:
        with (
            tc.tile_pool(name="const", bufs=1) as const_pool,
            tc.tile_pool(name="xin", bufs=6) as xin_pool,
            tc.tile_pool(name="outs", bufs=3) as out_pool,
            tc.tile_pool(name="psum_mm", bufs=4, space="PSUM") as psum_mm_pool,
        ):
            # resident folded weight, [i_within, i_tile, o]
            wtot = const_pool.tile([P, KB, O_LOC], BF16, name="wtot")
            w8_sb = const_pool.tile([P, NP8, 2, O_LOC], FP8, name="w8_sb")
            bias_sb = const_pool.tile([P, O_LOC], F32, name="bias_sb")

            def load_x(mt, chunks=1):
                x_tile = xin_pool.tile([P, KB, P], BF16, name="x_tile", tag="x_tile")
                x8_tile = xin_pool.tile(
                    [P, NP8, 2, P], FP8, name="x8_tile", tag="x8_tile"
                )
                if chunks == 1:
                    nc.sync.dma_start(x_tile[:], xt[mt])
                    nc.sync.dma_start(x8_tile[:], x8t[mt])
                    return x_tile, x8_tile, None
                # staged chunk sizes: tiny first slices unblock the
                # wavefront's opening matmuls ASAP
                bounds = (0, 2, 8, 16, KB)
                dmas = []
                for c in range(chunks):
                    lo, hi = bounds[c], bounds[c + 1]
                    dmas.append(
                        (x_tile[:, lo:hi, :], xt[mt, :, lo:hi, :])
                    )
                return x_tile, x8_tile, dmas

            def mm_step(x_tile, x8_tile, step, psum):
                """step 0..KB-1: bf16 k-tile; step KB..KB+NP8-1: fp8 pair."""
                if step < KB:
                    for os_ in range(OS):
                        nc.tensor.matmul(
                            psum[:, os_ * NF : (os_ + 1) * NF],
                            x_tile[:, step, :],
                            wtot[:, step, os_ * NF : (os_ + 1) * NF],
                            start=(step == 0),
                            stop=False,
                        )
                else:
                    p8 = step - KB
                    for os_ in range(OS):
                        nc.tensor.matmul(
                            psum[:, os_ * NF : (os_ + 1) * NF],
                            x8_tile[:, p8, :, :],
                            w8_sb[:, p8, :, os_ * NF : (os_ + 1) * NF],
                            start=False,
                            stop=(p8 == NP8 - 1),
                            perf_mode=mybir.MatmulPerfMode.DoubleRow,
                        )

            NSTEP = KB + NP8

            def store_out(mt, psum, split=False):
                if split:
                    # halve the drain latency on the final tile
                    for os_ in range(OS):
                        o_half = out_pool.tile(
                            [P, NF], BF16, name="o_half", tag="o_half"
                        )
                        nc.vector.tensor_add(
                            out=o_half[:],
                            in0=psum[:, os_ * NF : (os_ + 1) * NF],
                            in1=bias_sb[:, os_ * NF : (os_ + 1) * NF],
                        )
                        nc.scalar.dma_start(
                            out[mt * P : (mt + 1) * P, os_ * NF : (os_ + 1) * NF],
                            o_half[:],
                        )
                    return
                o_tile = out_pool.tile([P, O_LOC], BF16, name="o_tile", tag="o_tile")
                nc.vector.tensor_add(out=o_tile[:], in0=psum[:], in1=bias_sb[:])
                nc.scalar.dma_start(out[mt * P : (mt + 1) * P, :], o_tile[:])

            def new_psum():
                return psum_mm_pool.tile([P, OS * NF], F32, name="psum", tag="ps")

            # lead x tiles, chunked and interleaved so each tile's early k
            # slices land before the wavefront reaches them
            lead = [load_x(mt, chunks=XCH) for mt in range(NLEAD)]
            for c in range(XCH):
                for mt in range(NLEAD):
                    dst, src = lead[mt][2][c]
                    nc.sync.dma_start(dst, src)
            for mt in range(NLEAD):
                nc.sync.dma_start(lead[mt][1][:], x8t[mt])
            lead_psums = [new_psum() for _ in range(NLEAD)]

            # W stream: one DMA per k slice, in wavefront order, then fp8;
            # the first two slices arrive as os-halves so the wavefront's
            # opening matmuls unblock sooner. A tiny leading transfer wakes
            # the queue before the first real slice.
            nc.scalar.dma_start(bias_sb[0:1, :], br[0:1, :])
            for ki in range(KB):
                if ki < 2:
                    for os_ in range(OS):
                        nc.scalar.dma_start(
                            wtot[:, ki, os_ * NF : (os_ + 1) * NF],
                            wt[ki * P : (ki + 1) * P, os_ * NF : (os_ + 1) * NF],
                        )
                else:
                    nc.scalar.dma_start(wtot[:, ki, :], wt[ki * P : (ki + 1) * P, :])
            nc.scalar.dma_start(w8_sb[:], w8[:])
            nc.gpsimd.dma_start(bias_sb[:], br[:])

            # prefetch the next two x tiles behind the lead ones
            pre_x = [load_x(NLEAD + i) for i in range(2)]

            # ---- lead wavefront: 4 token tiles advance together through k,
            # staggered 2 k-steps apart so each tile's opening matmuls line
            # up with its x-chunk arrival
            SKEW = 2
            for s in range(NSTEP + SKEW * (NLEAD - 1)):
                for mt in range(NLEAD):
                    step = s - SKEW * mt
                    if 0 <= step < NSTEP:
                        mm_step(lead[mt][0], lead[mt][1], step, lead_psums[mt])
            for mt in range(NLEAD):
                store_out(mt, lead_psums[mt])

            # ---- steady state ----
            for mt in range(NLEAD, MT):
                idx = mt - NLEAD
                x_tile, x8_tile, _ = pre_x[idx] if idx < len(pre_x) else load_x(mt)
                psum = new_psum()
                for step in range(NSTEP):
                    mm_step(x_tile, x8_tile, step, psum)
                store_out(mt, psum, split=(mt == MT - 1))
    nc.finalize()
    return nc


def kernel(x, W, bias, lora_A, lora_B):
    x = np.asarray(x, dtype=np.float32)
    W = np.asarray(W, dtype=np.float32)
    bias = np.asarray(bias, dtype=np.float32)
    lora_A = np.asarray(lora_A, dtype=np.float32)
    lora_B = np.asarray(lora_B, dtype=np.float32)

    if "nc" not in _cache:
        _cache["nc"] = _build()
    nc = _cache["nc"]

    # fold the rank-16 LoRA delta into the weight while marshaling, and
    # pre-scale so fp8/bf16 partial products share one PSUM scale
    wts = (W + lora_A @ lora_B).T.astype(np.float32) * SW  # [in, out]

    xr = x.reshape(M_TOT, IN_F) * SX
    xb = xr[:, :K8].astype(ml_dtypes.bfloat16)
    x8 = np.asarray(np.clip(xr[:, K8:], -240, 240), dtype=E4M3)
    in_maps = []
    for c in range(8):
        mg, og = c % MG, c // MG
        sl = slice(mg * M_LOC, (mg + 1) * M_LOC)
        # [M_LOC, K8] -> (mt, m, ki, i) -> (mt, i, ki, m)
        xs = np.ascontiguousarray(
            xb[sl].reshape(MT, P, KB, P).transpose(0, 3, 2, 1)
        )
        # [M_LOC, 1024] -> (mt, m, pair, lane, i) -> (mt, i, pair, lane, m)
        xs8 = np.ascontiguousarray(
            x8[sl].reshape(MT, P, NP8, 2, P).transpose(0, 4, 2, 3, 1)
        )
        wcol = wts[:, og * O_LOC : (og + 1) * O_LOC]
        # [1024, O_LOC] -> (pair, lane, i, o) -> (i, pair, lane, o)
        w8s = np.ascontiguousarray(
            np.asarray(np.clip(wcol[K8:], -240, 240), dtype=E4M3)
            .reshape(NP8, 2, P, O_LOC)
            .transpose(2, 0, 1, 3)
        )
        in_maps.append(
            {
                "xt": xs,
                "x8t": xs8,
                "wt": np.ascontiguousarray(wcol[:K8].astype(ml_dtypes.bfloat16)),
                "w8": w8s,
                "br": np.ascontiguousarray(
                    np.broadcast_to(
                        bias[og * O_LOC : (og + 1) * O_LOC] * SOUT, (P, O_LOC)
                    )
                ).astype(np.float32),
            }
        )

    res = run_bass_kernel_spmd(nc, in_maps, core_ids=list(range(8)))

    out = np.empty((M_TOT, OUT_F), dtype=np.float32)
    inv = np.float32(1.0 / SOUT)
    for c in range(8):
        mg, og = c % MG, c // MG
        out[mg * M_LOC : (mg + 1) * M_LOC, og * O_LOC : (og + 1) * O_LOC] = (
            np.asarray(res.results[c]["out"], dtype=np.float32) * inv
        )
    return out.reshape(BATCH, SEQ, OUT_F)


# revision 20
# speedup vs baseline: 1.0082x; 1.0082x over previous
"""LoRA Linear kernel for Trainium2, 8 NeuronCores.

Computes out = x @ (W + lora_A @ lora_B)^T + bias for
x [4, 2048, 4096], W [4096, 4096], lora_A [4096, 16], lora_B [16, 4096].

Sharding: 2-way over tokens (M = 8192 -> 4096/core) x 4-way over
out_features (4096 -> 1024/core). The rank-16 LoRA delta is folded into
the weight during input marshaling (W_tot = W + A@B, 0.2% of the FLOPs;
the 274.9 GFLOP GEMM runs on device).

Mixed precision: the first 24 of 32 contraction tiles run in bf16; the
last 8 run as 4 double-pumped fp8(e4m3) DoubleRow matmuls (two k-tiles
per PE pass), cutting PE time ~11% while keeping rel_l2 ~1.9e-2.
Operands are pre-scaled on the host (x*16, W*4096, bias*65536 - exact
power-of-2 scales) so the e4m3 values sit mid-range and the fp8 and
bf16 partial products accumulate in one PSUM group; the 2^-16 descale
rides the host-side output upcast for free.

Device schedule: W_tot^T streams into a resident SBUF tile slice by
slice while the first four token tiles run as a k-major wavefront, so
the PE is saturated from ~13us on; the remaining 28 token tiles then
run back-to-back. PSUM tiles span two banks ([128, 1024] f32, 4 in
rotation = all 8 banks) so each token tile needs a single DVE bias-add
and a single 2KB-per-partition bf16 store.
"""

import ml_dtypes

import numpy as np

import concourse.bass as bass
import concourse.bacc as bacc
import concourse.mybir as mybir
import concourse.tile as tile
from concourse.bass_utils import run_bass_kernel_spmd

IN_F = 4096
OUT_F = 4096
RANK = 16
BATCH, SEQ = 4, 2048
M_TOT = BATCH * SEQ          # 8192 tokens
MG, OG = 2, 4                # shard grid: token-groups x outfeature-groups
M_LOC = M_TOT // MG          # 4096 tokens per core
O_LOC = OUT_F // OG          # 1024 out features per core
P = 128
KI = IN_F // P               # 32 contraction tiles
KB = 24                      # contraction tiles done in bf16
NP8 = (KI - KB) // 2         # fp8 DoubleRow pairs (4 -> 8 k-tiles)
K8 = KB * P                  # first fp8-region k index (3072)
NF = 512                     # matmul moving free dim (one PSUM bank)
OS = O_LOC // NF             # 2 output column passes
MT = M_LOC // P              # 32 token tiles per core
NLEAD = 4                    # token tiles in the k-major lead wavefront
XCH = 4                      # k-chunks per lead x-tile DMA
SX = 16.0                    # fp8 scale for x
SW = 4096.0                  # fp8 scale for W
SOUT = SX * SW               # psum / bias / output scale

F32 = mybir.dt.float32
BF16 = mybir.dt.bfloat16
FP8 = mybir.dt.float8e4
E4M3 = ml_dtypes.float8_e4m3

_cache = {}


def _build():
    nc = bacc.Bacc(None, target_bir_lowering=False)

    # x pre-tiled on host to [MT, P, KB, P]: (mt, i_within, i_tile, m)
    xt = nc.dram_tensor("xt", [MT, P, KB, P], BF16, kind="ExternalInput")
    # fp8 lanes: (mt, i_within, pair, lane, m)
    x8t = nc.dram_tensor("x8t", [MT, P, NP8, 2, P], FP8, kind="ExternalInput")
    # pre-folded, pre-scaled (W + lora_A @ lora_B)^T column shard
    wt = nc.dram_tensor("wt", [KB * P, O_LOC], BF16, kind="ExternalInput")
    w8 = nc.dram_tensor("w8", [P, NP8, 2, O_LOC], FP8, kind="ExternalInput")
    br = nc.dram_tensor("br", [P, O_LOC], F32, kind="ExternalInput")
    out = nc.dram_tensor("out", [M_LOC, O_LOC], BF16, kind="ExternalOutput")

    with tile.TileContext(nc) as tc:
        with (
            tc.tile_pool(name="const", bufs=1) as const_pool,
            tc.tile_pool(name="xin", bufs=6) as xin_pool,
            tc.tile_pool(name="outs", bufs=3) as out_pool,
            tc.tile_pool(name="psum_mm", bufs=4, space="PSUM") as psum_mm_pool,
        ):
            # resident folded weight, [i_within, i_tile, o]
            wtot = const_pool.tile([P, KB, O_LOC], BF16, name="wtot")
            w8_sb = const_pool.tile([P, NP8, 2, O_LOC], FP8, name="w8_sb")
            bias_sb = const_pool.tile([P, O_LOC], F32, name="bias_sb")

            def load_x(mt, chunks=1):
                x_tile = xin_pool.tile([P, KB, P], BF16, name="x_tile", tag="x_tile")
                x8_tile = xin_pool.tile(
                    [P, NP8, 2, P], FP8, name="x8_tile", tag="x8_tile"
                )
                if chunks == 1:
                    nc.sync.dma_start(x_tile[:], xt[mt])
                    nc.sync.dma_start(x8_tile[:], x8t[mt])
                    return x_tile, x8_tile, None
                # staged chunk sizes: tiny first slices unblock the
                # wavefront's opening matmuls ASAP
                bounds = (0, 2, 8, 16, KB)
                dmas = []
                for c in range(chunks):
                    lo, hi = bounds[c], bounds[c + 1]
                    dmas.append(
                        (x_tile[:, lo:hi, :], xt[mt, :, lo:hi, :])
                    )
                return x_tile, x8_tile, dmas

            def mm_step(x_tile, x8_tile, step, psum):
                """step 0..KB-1: bf16 k-tile; step KB..KB+NP8-1: fp8 pair."""
                if step < KB:
                    for os_ in range(OS):
                        nc.tensor.matmul(
                            psum[:, os_ * NF : (os_ + 1) * NF],
                            x_tile[:, step, :],
                            wtot[:, step, os_ * NF : (os_ + 1) * NF],
                            start=(step == 0),
                            stop=False,
                        )
                else:
                    p8 = step - KB
                    for os_ in range(OS):
                        nc.tensor.matmul(
                            psum[:, os_ * NF : (os_ + 1) * NF],
                            x8_tile[:, p8, :, :],
                            w8_sb[:, p8, :, os_ * NF : (os_ + 1) * NF],
                            start=False,
                            stop=(p8 == NP8 - 1),
                            perf_mode=mybir.MatmulPerfMode.DoubleRow,
                        )

            NSTEP = KB + NP8

            def store_out(mt, psum, split=False):
                if split:
                    # halve the drain latency on the final tile
                    for os_ in range(OS):
                        o_half = out_pool.tile(
                            [P, NF], BF16, name="o_half", tag="o_half"
                        )
                        nc.vector.tensor_add(
                            out=o_half[:],
                            in0=psum[:, os_ * NF : (os_ + 1) * NF],
                            in1=bias_sb[:, os_ * NF : (os_ + 1) * NF],
                        )
                        nc.scalar.dma_start(
                            out[mt * P : (mt + 1) * P, os_ * NF : (os_ + 1) * NF],
                            o_half[:],
                        )
                    return
                o_tile = out_pool.tile([P, O_LOC], BF16, name="o_tile", tag="o_tile")
                nc.vector.tensor_add(out=o_tile[:], in0=psum[:], in1=bias_sb[:])
                nc.scalar.dma_start(out[mt * P : (mt + 1) * P, :], o_tile[:])

            def new_psum():
                return psum_mm_pool.tile([P, OS * NF], F32, name="psum", tag="ps")

            # lead x tiles, chunked and interleaved so each tile's early k
            # slices land before the wavefront reaches them
            lead = [load_x(mt, chunks=XCH) for mt in range(NLEAD)]
            for c in range(XCH):
                for mt in range(NLEAD):
                    dst, src = lead[mt][2][c]
                    nc.sync.dma_start(dst, src)
            for mt in range(NLEAD):
                nc.sync.dma_start(lead[mt][1][:], x8t[mt])
            lead_psums = [new_psum() for _ in range(NLEAD)]

            # W stream: one DMA per k slice, in wavefront order, then fp8;
            # the first two slices arrive as os-halves so the wavefront's
            # opening matmuls unblock sooner. A tiny leading transfer wakes
            # the queue before the first real slice.
            nc.scalar.dma_start(bias_sb[0:1, :], br[0:1, :])
            for ki in range(KB):
                if ki < 2:
                    for os_ in range(OS):
                        nc.scalar.dma_start(
                            wtot[:, ki, os_ * NF : (os_ + 1) * NF],
                            wt[ki * P : (ki + 1) * P, os_ * NF : (os_ + 1) * NF],
                        )
                else:
                    nc.scalar.dma_start(wtot[:, ki, :], wt[ki * P : (ki + 1) * P, :])
            nc.scalar.dma_start(w8_sb[:], w8[:])
            nc.gpsimd.dma_start(bias_sb[:], br[:])

            # prefetch the next two x tiles behind the lead ones
            pre_x = [load_x(NLEAD + i) for i in range(2)]

            # ---- lead wavefront: 4 token tiles advance together through k
            for step in range(NSTEP):
                for mt in range(NLEAD):
                    mm_step(lead[mt][0], lead[mt][1], step, lead_psums[mt])
            for mt in range(NLEAD):
                store_out(mt, lead_psums[mt])

            # ---- steady state ----
            for mt in range(NLEAD, MT):
                idx = mt - NLEAD
                x_tile, x8_tile, _ = pre_x[idx] if idx < len(pre_x) else load_x(mt)
                psum = new_psum()
                for step in range(NSTEP):
                    mm_step(x_tile, x8_tile, step, psum)
                store_out(mt, psum, split=(mt == MT - 1))
    nc.finalize()
    return nc


def kernel(x, W, bias, lora_A, lora_B):
    x = np.asarray(x, dtype=np.float32)
    W = np.asarray(W, dtype=np.float32)
    bias = np.asarray(bias, dtype=np.float32)
    lora_A = np.asarray(lora_A, dtype=np.float32)
    lora_B = np.asarray(lora_B, dtype=np.float32)

    if "nc" not in _cache:
        _cache["nc"] = _build()
    nc = _cache["nc"]

    # fold the rank-16 LoRA delta into the weight while marshaling, and
    # pre-scale so fp8/bf16 partial products share one PSUM scale
    wts = (W + lora_A @ lora_B).T.astype(np.float32) * SW  # [in, out]

    xr = x.reshape(M_TOT, IN_F) * SX
    xb = xr[:, :K8].astype(ml_dtypes.bfloat16)
    x8 = np.asarray(np.clip(xr[:, K8:], -240, 240), dtype=E4M3)
    in_maps = []
    for c in range(8):
        mg, og = c % MG, c // MG
        sl = slice(mg * M_LOC, (mg + 1) * M_LOC)
        # [M_LOC, K8] -> (mt, m, ki, i) -> (mt, i, ki, m)
        xs = np.ascontiguousarray(
            xb[sl].reshape(MT, P, KB, P).transpose(0, 3, 2, 1)
        )
        # [M_LOC, 1024] -> (mt, m, pair, lane, i) -> (mt, i, pair, lane, m)
        xs8 = np.ascontiguousarray(
            x8[sl].reshape(MT, P, NP8, 2, P).transpose(0, 4, 2, 3, 1)
        )
        wcol = wts[:, og * O_LOC : (og + 1) * O_LOC]
        # [1024, O_LOC] -> (pair, lane, i, o) -> (i, pair, lane, o)
        w8s = np.ascontiguousarray(
            np.asarray(np.clip(wcol[K8:], -240, 240), dtype=E4M3)
            .reshape(NP8, 2, P, O_LOC)
            .transpose(2, 0, 1, 3)
        )
        in_maps.append(
            {
                "xt": xs,
                "x8t": xs8,
                "wt": np.ascontiguousarray(wcol[:K8].astype(ml_dtypes.bfloat16)),
                "w8": w8s,
                "br": np.ascontiguousarray(
                    np.broadcast_to(
                        bias[og * O_LOC : (og + 1) * O_LOC] * SOUT, (P, O_LOC)
                    )
                ).astype(np.float32),
            }
        )

    res = run_bass_kernel_spmd(nc, in_maps, core_ids=list(range(8)))

    out = np.empty((M_TOT, OUT_F), dtype=np.float32)
    inv = np.float32(1.0 / SOUT)
    for c in range(8):
        mg, og = c % MG, c // MG
        out[mg * M_LOC : (mg + 1) * M_LOC, og * O_LOC : (og + 1) * O_LOC] = (
            np.asarray(res.results[c]["out"], dtype=np.float32) * inv
        )
    return out.reshape(BATCH, SEQ, OUT_F)


# revision 21
# speedup vs baseline: 1.0148x; 1.0066x over previous
"""LoRA Linear kernel for Trainium2, 8 NeuronCores.

Computes out = x @ (W + lora_A @ lora_B)^T + bias for
x [4, 2048, 4096], W [4096, 4096], lora_A [4096, 16], lora_B [16, 4096].

Sharding: 2-way over tokens (M = 8192 -> 4096/core) x 4-way over
out_features (4096 -> 1024/core). The rank-16 LoRA delta is folded into
the weight during input marshaling (W_tot = W + A@B, 0.2% of the FLOPs;
the 274.9 GFLOP GEMM runs on device).

Mixed precision: the first 24 of 32 contraction tiles run in bf16; the
last 8 run as 4 double-pumped fp8(e4m3) DoubleRow matmuls (two k-tiles
per PE pass), cutting PE time ~11% while keeping rel_l2 ~1.9e-2.
Operands are pre-scaled on the host (x*16, W*4096, bias*65536 - exact
power-of-2 scales) so the e4m3 values sit mid-range and the fp8 and
bf16 partial products accumulate in one PSUM group; the 2^-16 descale
rides the host-side output upcast for free.

Device schedule: W_tot^T streams into a resident SBUF tile slice by
slice while the first four token tiles run as a k-major wavefront, so
the PE is saturated from ~13us on; the remaining 28 token tiles then
run back-to-back. PSUM tiles span two banks ([128, 1024] f32, 4 in
rotation = all 8 banks) so each token tile needs a single DVE bias-add
and a single 2KB-per-partition bf16 store.
"""

import ml_dtypes

import numpy as np

import concourse.bass as bass
import concourse.bacc as bacc
import concourse.mybir as mybir
import concourse.tile as tile
from concourse.bass_utils import run_bass_kernel_spmd

IN_F = 4096
OUT_F = 4096
RANK = 16
BATCH, SEQ = 4, 2048
M_TOT = BATCH * SEQ          # 8192 tokens
MG, OG = 2, 4                # shard grid: token-groups x outfeature-groups
M_LOC = M_TOT // MG          # 4096 tokens per core
O_LOC = OUT_F // OG          # 1024 out features per core
P = 128
KI = IN_F // P               # 32 contraction tiles
KB = 24                      # contraction tiles done in bf16
NP8 = (KI - KB) // 2         # fp8 DoubleRow pairs (4 -> 8 k-tiles)
K8 = KB * P                  # first fp8-region k index (3072)
NF = 512                     # matmul moving free dim (one PSUM bank)
OS = O_LOC // NF             # 2 output column passes
MT = M_LOC // P              # 32 token tiles per core
NLEAD = 4                    # token tiles in the k-major lead wavefront
XCH = 4                      # k-chunks per lead x-tile DMA
SX = 16.0                    # fp8 scale for x
SW = 4096.0                  # fp8 scale for W
SOUT = SX * SW               # psum / bias / output scale

F32 = mybir.dt.float32
BF16 = mybir.dt.bfloat16
FP8 = mybir.dt.float8e4
E4M3 = ml_dtypes.float8_e4m3

_cache = {}


def _build():
    nc = bacc.Bacc(None, target_bir_lowering=False)

    # x pre-tiled on host to [MT, P, KB, P]: (mt, i_within, i_tile, m)
    xt = nc.dram_tensor("xt", [MT, P, KB, P], BF16, kind="ExternalInput")
    # fp8 lanes: (mt, i_within, pair, lane, m)
    x8t = nc.dram_tensor("x8t", [MT, P, NP8, 2, P], FP8, kind="ExternalInput")
    # pre-folded, pre-scaled (W + lora_A @ lora_B)^T column shard
    wt = nc.dram_tensor("wt", [KB * P, O_LOC], BF16, kind="ExternalInput")
    w8 = nc.dram_tensor("w8", [P, NP8, 2, O_LOC], FP8, kind="ExternalInput")
    br = nc.dram_tensor("br", [P, O_LOC], F32, kind="ExternalInput")
    out = nc.dram_tensor("out", [M_LOC, O_LOC], BF16, kind="ExternalOutput")

    with tile.TileContext(nc) as tc:
        with (
            tc.tile_pool(name="const", bufs=1) as const_pool,
            tc.tile_pool(name="xin", bufs=6) as xin_pool,
            tc.tile_pool(name="outs", bufs=3) as out_pool,
            tc.tile_pool(name="psum_mm", bufs=4, space="PSUM") as psum_mm_pool,
        ):
            # resident folded weight, [i_within, i_tile, o]
            wtot = const_pool.tile([P, KB, O_LOC], BF16, name="wtot")
            w8_sb = const_pool.tile([P, NP8, 2, O_LOC], FP8, name="w8_sb")
            bias_sb = const_pool.tile([P, O_LOC], F32, name="bias_sb")

            def load_x(mt, chunks=1):
                x_tile = xin_pool.tile([P, KB, P], BF16, name="x_tile", tag="x_tile")
                x8_tile = xin_pool.tile(
                    [P, NP8, 2, P], FP8, name="x8_tile", tag="x8_tile"
                )
                if chunks == 1:
                    nc.sync.dma_start(x_tile[:], xt[mt])
                    nc.sync.dma_start(x8_tile[:], x8t[mt])
                    return x_tile, x8_tile, None
                # staged chunk sizes: tiny first slices unblock the
                # wavefront's opening matmuls ASAP
                bounds = (0, 2, 8, 16, KB)
                dmas = []
                for c in range(chunks):
                    lo, hi = bounds[c], bounds[c + 1]
                    dmas.append(
                        (x_tile[:, lo:hi, :], xt[mt, :, lo:hi, :])
                    )
                return x_tile, x8_tile, dmas

            def mm_step(x_tile, x8_tile, step, psum):
                """step 0..KB-1: bf16 k-tile; step KB..KB+NP8-1: fp8 pair."""
                if step < KB:
                    for os_ in range(OS):
                        nc.tensor.matmul(
                            psum[:, os_ * NF : (os_ + 1) * NF],
                            x_tile[:, step, :],
                            wtot[:, step, os_ * NF : (os_ + 1) * NF],
                            start=(step == 0),
                            stop=False,
                        )
                else:
                    p8 = step - KB
                    for os_ in range(OS):
                        nc.tensor.matmul(
                            psum[:, os_ * NF : (os_ + 1) * NF],
                            x8_tile[:, p8, :, :],
                            w8_sb[:, p8, :, os_ * NF : (os_ + 1) * NF],
                            start=False,
                            stop=(p8 == NP8 - 1),
                            perf_mode=mybir.MatmulPerfMode.DoubleRow,
                        )

            NSTEP = KB + NP8

            def store_out(mt, psum, split=False):
                if split:
                    # halve the drain latency on the final tile
                    for os_ in range(OS):
                        o_half = out_pool.tile(
                            [P, NF], BF16, name="o_half", tag="o_half"
                        )
                        nc.vector.tensor_add(
                            out=o_half[:],
                            in0=psum[:, os_ * NF : (os_ + 1) * NF],
                            in1=bias_sb[:, os_ * NF : (os_ + 1) * NF],
                        )
                        nc.scalar.dma_start(
                            out[mt * P : (mt + 1) * P, os_ * NF : (os_ + 1) * NF],
                            o_half[:],
                        )
                    return
                o_tile = out_pool.tile([P, O_LOC], BF16, name="o_tile", tag="o_tile")
                nc.vector.tensor_add(out=o_tile[:], in0=psum[:], in1=bias_sb[:])
                nc.scalar.dma_start(out[mt * P : (mt + 1) * P, :], o_tile[:])

            def new_psum():
                return psum_mm_pool.tile([P, OS * NF], F32, name="psum", tag="ps")

            # lead x tiles, chunked and interleaved so each tile's early k
            # slices land before the wavefront reaches them
            lead = [load_x(mt, chunks=XCH) for mt in range(NLEAD)]
            for c in range(XCH):
                for mt in range(NLEAD):
                    dst, src = lead[mt][2][c]
                    nc.sync.dma_start(dst, src)
            for mt in range(NLEAD):
                nc.sync.dma_start(lead[mt][1][:], x8t[mt])
            lead_psums = [new_psum() for _ in range(NLEAD)]

            # W stream: one DMA per k slice, in wavefront order, then fp8;
            # the first two slices arrive as os-halves so the wavefront's
            # opening matmuls unblock sooner
            for ki in range(KB):
                if ki < 2:
                    for os_ in range(OS):
                        nc.scalar.dma_start(
                            wtot[:, ki, os_ * NF : (os_ + 1) * NF],
                            wt[ki * P : (ki + 1) * P, os_ * NF : (os_ + 1) * NF],
                        )
                else:
                    nc.scalar.dma_start(wtot[:, ki, :], wt[ki * P : (ki + 1) * P, :])
            nc.scalar.dma_start(w8_sb[:], w8[:])
            nc.gpsimd.dma_start(bias_sb[:], br[:])

            # prefetch the next two x tiles behind the lead ones
            pre_x = [load_x(NLEAD + i) for i in range(2)]

            # ---- lead wavefront: 4 token tiles advance together through k
            for step in range(NSTEP):
                for mt in range(NLEAD):
                    mm_step(lead[mt][0], lead[mt][1], step, lead_psums[mt])
            for mt in range(NLEAD):
                store_out(mt, lead_psums[mt])

            # ---- steady state ----
            for mt in range(NLEAD, MT):
                idx = mt - NLEAD
                x_tile, x8_tile, _ = pre_x[idx] if idx < len(pre_x) else load_x(mt)
                psum = new_psum()
                for step in range(NSTEP):
                    mm_step(x_tile, x8_tile, step, psum)
                store_out(mt, psum, split=(mt == MT - 1))
    nc.finalize()
    return nc


def kernel(x, W, bias, lora_A, lora_B):
    x = np.asarray(x, dtype=np.float32)
    W = np.asarray(W, dtype=np.float32)
    bias = np.asarray(bias, dtype=np.float32)
    lora_A = np.asarray(lora_A, dtype=np.float32)
    lora_B = np.asarray(lora_B, dtype=np.float32)

    if "nc" not in _cache:
        _cache["nc"] = _build()
    nc = _cache["nc"]

    # fold the rank-16 LoRA delta into the weight while marshaling, and
    # pre-scale so fp8/bf16 partial products share one PSUM scale
    wts = (W + lora_A @ lora_B).T.astype(np.float32) * SW  # [in, out]

    xr = x.reshape(M_TOT, IN_F) * SX
    xb = xr[:, :K8].astype(ml_dtypes.bfloat16)
    x8 = np.asarray(np.clip(xr[:, K8:], -240, 240), dtype=E4M3)
    in_maps = []
    for c in range(8):
        mg, og = c % MG, c // MG
        sl = slice(mg * M_LOC, (mg + 1) * M_LOC)
        # [M_LOC, K8] -> (mt, m, ki, i) -> (mt, i, ki, m)
        xs = np.ascontiguousarray(
            xb[sl].reshape(MT, P, KB, P).transpose(0, 3, 2, 1)
        )
        # [M_LOC, 1024] -> (mt, m, pair, lane, i) -> (mt, i, pair, lane, m)
        xs8 = np.ascontiguousarray(
            x8[sl].reshape(MT, P, NP8, 2, P).transpose(0, 4, 2, 3, 1)
        )
        wcol = wts[:, og * O_LOC : (og + 1) * O_LOC]
        # [1024, O_LOC] -> (pair, lane, i, o) -> (i, pair, lane, o)
        w8s = np.ascontiguousarray(
            np.asarray(np.clip(wcol[K8:], -240, 240), dtype=E4M3)
            .reshape(NP8, 2, P, O_LOC)
            .transpose(2, 0, 1, 3)
        )
        in_maps.append(
            {
                "xt": xs,
                "x8t": xs8,
                "wt": np.ascontiguousarray(wcol[:K8].astype(ml_dtypes.bfloat16)),
                "w8": w8s,
                "br": np.ascontiguousarray(
                    np.broadcast_to(
                        bias[og * O_LOC : (og + 1) * O_LOC] * SOUT, (P, O_LOC)
                    )
                ).astype(np.float32),
            }
        )

    res = run_bass_kernel_spmd(nc, in_maps, core_ids=list(range(8)))

    out = np.empty((M_TOT, OUT_F), dtype=np.float32)
    inv = np.float32(1.0 / SOUT)
    for c in range(8):
        mg, og = c % MG, c // MG
        out[mg * M_LOC : (mg + 1) * M_LOC, og * O_LOC : (og + 1) * O_LOC] = (
            np.asarray(res.results[c]["out"], dtype=np.float32) * inv
        )
    return out.reshape(BATCH, SEQ, OUT_F)
